# revision 2
# baseline (speedup 1.0000x reference)
"""Trainium2 Bass kernel for NeuralODETrajectory.

Math: reference integrates y' = y @ W.T + b with dopri5, 2 fixed substeps of
h=0.5 per interval, 31 intervals. For b == 0 the dynamics are linear, so each
substep is y <- y @ R(z) with z = 0.5 * W.T and R the dopri5 stability
polynomial (coeffs 1, 1, 1/2, 1/6, 1/24, 1/120, 1/600). Per interval:
y <- y @ R(z)^2 = y + y @ E2, with E2 = R(z)^2 - I truncated at degree 2:
    E2 = 2z + 2z^2 = u + 0.5*u^2,  u = W.T
(verified: degree-2 truncation error 4.8e-6 scale-relative; full f32r-rounded
pipeline proxy 8.3e-5).

Sharding: data-parallel over the batch dim — 128 rows of start_embedding per
core, W replicated. Device computes E2 once, then 31 iterations of
y += y @ E2 with f32r matmuls / fp32 state, streaming each state to DRAM.
"""

import numpy as np

D = 1024
NB = D // 128          # 8 blocks of 128
N_CORES = 8
ROWS = D // N_CORES    # 128 batch rows per core
STEPS = 31             # intervals (output rows 1..31; row 0 is y0)

_CACHE = {}


def _build():
    import concourse.bacc as bacc
    import concourse.mybir as mybir
    from concourse import tile, masks

    f32 = mybir.dt.float32
    f32r = mybir.dt.float32r

    nc = bacc.Bacc("TRN2", target_bir_lowering=False, debug=False,
                   num_devices=N_CORES)
    y0 = nc.dram_tensor("y0", [ROWS, D], f32, kind="ExternalInput").ap()
    w = nc.dram_tensor("w", [D, D], f32, kind="ExternalInput").ap()
    out = nc.dram_tensor("out", [STEPS * ROWS, D], f32,
                         kind="ExternalOutput").ap()

    with tile.TileContext(nc) as tc:
        with tc.tile_pool(name="sbuf", bufs=1) as pool, \
             tc.tile_pool(name="psum", bufs=1, space="PSUM") as psum:
            ident = pool.tile([128, 128], f32, tag="ident")
            masks.make_identity(nc, ident[:])

            # --- load W, rounded 0.5*W (stationary for u^2) ---
            w_sb = [pool.tile([128, D], f32, tag=f"w{j}", name=f"w{j}") for j in range(NB)]
            for j in range(NB):
                nc.sync.dma_start(out=w_sb[j][:], in_=w[j*128:(j+1)*128, :])
            wh_r = [pool.tile([128, D], f32r, tag=f"wh{j}", name=f"wh{j}") for j in range(NB)]
            for j in range(NB):
                nc.vector.tensor_scalar_mul(wh_r[j][:], w_sb[j][:], 0.5)

            # --- u = W.T via PE transposes ---
            u_sb = [pool.tile([128, D], f32, tag=f"u{i}", name=f"u{i}") for i in range(NB)]
            u_r = [pool.tile([128, D], f32r, tag=f"ur{i}", name=f"ur{i}") for i in range(NB)]
            tp = psum.tile([128, D], f32, tag="tp")
            for i in range(NB):
                for j in range(NB):
                    nc.tensor.transpose(tp[:, j*128:(j+1)*128],
                                        w_sb[j][:, i*128:(i+1)*128], ident[:])
                nc.vector.tensor_copy(u_sb[i][:], tp[:])
                nc.vector.tensor_copy(u_r[i][:], tp[:])

            # --- E2 = u + 0.5*u^2 (block row i at a time) ---
            e2_r = [pool.tile([128, D], f32r, tag=f"e2{i}", name=f"e2{i}") for i in range(NB)]
            pacc = psum.tile([128, D], f32, tag="pacc")
            for i in range(NB):
                for m in range(NB):
                    for n in range(2):
                        nc.tensor.matmul(pacc[:, n*512:(n+1)*512],
                                         wh_r[m][:, i*128:(i+1)*128],
                                         u_r[m][:, n*512:(n+1)*512],
                                         start=(m == 0), stop=(m == NB - 1))
                nc.vector.tensor_tensor(e2_r[i][:], u_sb[i][:], pacc[:],
                                        op=mybir.AluOpType.add)

            # --- main loop: y += y @ E2, 31 times ---
            y_a = pool.tile([128, D], f32, tag="y_a")
            y_b = pool.tile([128, D], f32, tag="y_b")
            nc.sync.dma_start(out=y_a[:], in_=y0)
            yT_r = [pool.tile([128, 128], f32r, tag=f"yT{k}", name=f"yT{k}")
                    for k in range(NB)]
            ys = [y_a, y_b]
            for it in range(STEPS):
                y_cur = ys[it % 2]
                y_nxt = ys[(it + 1) % 2]
                for k in range(NB):
                    nc.tensor.transpose(tp[:, k*128:(k+1)*128],
                                        y_cur[:, k*128:(k+1)*128], ident[:])
                for k in range(NB):
                    nc.vector.tensor_copy(yT_r[k][:], tp[:, k*128:(k+1)*128])
                for k in range(NB):
                    for n in range(2):
                        nc.tensor.matmul(pacc[:, n*512:(n+1)*512],
                                         yT_r[k][:],
                                         e2_r[k][:, n*512:(n+1)*512],
                                         start=(k == 0), stop=(k == NB - 1))
                nc.vector.tensor_tensor(y_nxt[:], y_cur[:], pacc[:],
                                        op=mybir.AluOpType.add)
                nc.sync.dma_start(out=out[it*ROWS:(it+1)*ROWS, :],
                                  in_=y_nxt[:])

    nc.compile()
    return nc


def _get_nc():
    nc = _CACHE.get("nc")
    if nc is None:
        nc = _build()
        _CACHE["nc"] = nc
    return nc


def _make_in_maps(start_embedding, W):
    return [{"y0": np.ascontiguousarray(start_embedding[c*ROWS:(c+1)*ROWS, :]),
             "w": W} for c in range(N_CORES)]


def _assemble(start_embedding, results):
    out = np.empty((STEPS + 1, D, D), dtype=np.float32)
    out[0] = start_embedding
    for c in range(N_CORES):
        out[1:, c*ROWS:(c+1)*ROWS, :] = \
            results[c]["out"].reshape(STEPS, ROWS, D)
    return out


def _dopri5_step(y, h, M, b):
    def f(v):
        return v @ M + b
    k1 = f(y)
    k2 = f(y + h * (1.0/5.0) * k1)
    k3 = f(y + h * (3.0/40.0*k1 + 9.0/40.0*k2))
    k4 = f(y + h * (44.0/45.0*k1 - 56.0/15.0*k2 + 32.0/9.0*k3))
    k5 = f(y + h * (19372.0/6561.0*k1 - 25360.0/2187.0*k2
                    + 64448.0/6561.0*k3 - 212.0/729.0*k4))
    k6 = f(y + h * (9017.0/3168.0*k1 - 355.0/33.0*k2 + 46732.0/5247.0*k3
                    + 49.0/176.0*k4 - 5103.0/18656.0*k5))
    return y + h * (35.0/384.0*k1 + 500.0/1113.0*k3 + 125.0/192.0*k4
                    - 2187.0/6784.0*k5 + 11.0/84.0*k6)


def _fallback(start_embedding, t_eval, W, b):
    M = W.T.astype(np.float64)
    bb = np.asarray(b, dtype=np.float64)
    y = start_embedding.astype(np.float64)
    t = np.asarray(t_eval, dtype=np.float64)
    traj = [y.copy()]
    for k in range(t.shape[0] - 1):
        h = (t[k+1] - t[k]) / 2.0
        for _ in range(2):
            y = _dopri5_step(y, h, M, bb)
        traj.append(y.copy())
    return np.stack(traj).astype(np.float32)


def kernel(start_embedding, t_eval, W, b):
    start_embedding = np.ascontiguousarray(start_embedding, dtype=np.float32)
    W32 = np.ascontiguousarray(W, dtype=np.float32)
    t = np.asarray(t_eval, dtype=np.float64)
    fast_ok = (start_embedding.shape == (D, D) and W32.shape == (D, D)
               and t.shape == (32,)
               and np.array_equal(t, np.arange(32, dtype=np.float64))
               and not np.any(np.asarray(b)))
    if not fast_ok:
        return _fallback(start_embedding, t_eval, W32, np.asarray(b))

    from concourse.bass_utils import run_bass_kernel_spmd
    nc = _get_nc()
    in_maps = _make_in_maps(start_embedding, W32)
    res = run_bass_kernel_spmd(nc, in_maps, list(range(N_CORES)))
    return _assemble(start_embedding, res.results)


# revision 3
# speedup vs baseline: 1.6536x; 1.6536x over previous
"""Trainium2 Bass kernel for NeuralODETrajectory.

Math: reference integrates y' = y @ W.T + b with dopri5, 2 fixed substeps of
h=0.5 per interval, 31 intervals. For b == 0 the dynamics are linear: one
substep is y <- y @ S with S = dopri5_step(I). The host computes the exact
(f64) two-interval propagator delta E4 = S^4 - I and the interval-1 state
y1 = y0 @ S^2, so the device only runs the recurrence y <- y + y @ E4.

Device: two independent chains per core (even intervals seeded by y0, odd
intervals seeded by y1), interleaved so one chain's add/cast latency hides
under the other chain's matmuls. f32r matmuls, fp32 state, 15 steps/chain.

Sharding: data-parallel over the batch dim - 128 rows per core, E4 replicated.
"""

import numpy as np

D = 1024
NB = D // 128          # 8 blocks of 128
N_CORES = 8
ROWS = D // N_CORES    # 128 batch rows per core
CHAIN_STEPS = 15       # steps per chain; 2 chains -> intervals 2..31
OUT_BLOCKS = 2 * CHAIN_STEPS

_CACHE = {}


def _build():
    import concourse.bacc as bacc
    import concourse.mybir as mybir
    from concourse import tile, masks

    f32 = mybir.dt.float32
    f32r = mybir.dt.float32r

    nc = bacc.Bacc("TRN2", target_bir_lowering=False, debug=False,
                   num_devices=N_CORES)
    ya0 = nc.dram_tensor("ya0", [ROWS, D], f32, kind="ExternalInput").ap()
    yb0 = nc.dram_tensor("yb0", [ROWS, D], f32, kind="ExternalInput").ap()
    e4 = nc.dram_tensor("e4", [D, D], f32r, kind="ExternalInput").ap()
    out = nc.dram_tensor("out", [OUT_BLOCKS * ROWS, D], f32,
                         kind="ExternalOutput").ap()

    with tile.TileContext(nc) as tc:
        with tc.tile_pool(name="sbuf", bufs=1) as pool, \
             tc.tile_pool(name="psum", bufs=1, space="PSUM") as psum:
            ident = pool.tile([128, 128], f32, tag="ident")
            masks.make_identity(nc, ident[:])

            e4_sb = [pool.tile([128, D], f32r, tag=f"e4_{k}", name=f"e4_{k}")
                     for k in range(NB)]
            y = {c: [pool.tile([128, D], f32, tag=f"y{c}{i}", name=f"y{c}{i}")
                     for i in range(2)] for c in "ab"}
            yT = {c: [pool.tile([128, 128], f32r, tag=f"yT{c}{k}",
                                name=f"yT{c}{k}") for k in range(NB)]
                  for c in "ab"}
            tp = {c: psum.tile([128, D], f32, tag=f"tp_{c}", name=f"tp_{c}")
                  for c in "ab"}
            pacc = {c: psum.tile([128, D], f32, tag=f"pacc_{c}",
                                 name=f"pacc_{c}") for c in "ab"}

            nc.sync.dma_start(out=y["a"][0][:], in_=ya0)
            nc.sync.dma_start(out=y["b"][0][:], in_=yb0)
            for k in range(NB):
                nc.sync.dma_start(out=e4_sb[k][:],
                                  in_=e4[k*128:(k+1)*128, :])

            for s in range(CHAIN_STEPS):
                for ci, c in enumerate("ab"):
                    y_cur = y[c][s % 2]
                    y_nxt = y[c][(s + 1) % 2]
                    for k in range(NB):
                        nc.tensor.transpose(tp[c][:, k*128:(k+1)*128],
                                            y_cur[:, k*128:(k+1)*128],
                                            ident[:])
                    for k in range(NB):
                        nc.vector.tensor_copy(yT[c][k][:],
                                              tp[c][:, k*128:(k+1)*128])
                    for k in range(NB):
                        for n in range(2):
                            nc.tensor.matmul(pacc[c][:, n*512:(n+1)*512],
                                             yT[c][k][:],
                                             e4_sb[k][:, n*512:(n+1)*512],
                                             start=(k == 0), stop=(k == NB - 1))
                    nc.vector.tensor_tensor(y_nxt[:], y_cur[:], pacc[c][:],
                                            op=mybir.AluOpType.add)
                    idx = 2 * s + ci
                    nc.sync.dma_start(out=out[idx*ROWS:(idx+1)*ROWS, :],
                                      in_=y_nxt[:])

    nc.compile()
    return nc


def _get_nc():
    nc = _CACHE.get("nc")
    if nc is None:
        nc = _build()
        _CACHE["nc"] = nc
    return nc


def _dopri5_step(y, h, M, b):
    def f(v):
        return v @ M + b
    k1 = f(y)
    k2 = f(y + h * (1.0/5.0) * k1)
    k3 = f(y + h * (3.0/40.0*k1 + 9.0/40.0*k2))
    k4 = f(y + h * (44.0/45.0*k1 - 56.0/15.0*k2 + 32.0/9.0*k3))
    k5 = f(y + h * (19372.0/6561.0*k1 - 25360.0/2187.0*k2
                    + 64448.0/6561.0*k3 - 212.0/729.0*k4))
    k6 = f(y + h * (9017.0/3168.0*k1 - 355.0/33.0*k2 + 46732.0/5247.0*k3
                    + 49.0/176.0*k4 - 5103.0/18656.0*k5))
    return y + h * (35.0/384.0*k1 + 500.0/1113.0*k3 + 125.0/192.0*k4
                    - 2187.0/6784.0*k5 + 11.0/84.0*k6)


def _host_propagators(W32):
    M = W32.T.astype(np.float64)
    S = _dopri5_step(np.eye(D), 0.5, M, 0.0)
    A = S @ S                       # one-interval propagator
    E4 = A @ A - np.eye(D)          # two-interval delta
    return A, np.ascontiguousarray(E4.astype(np.float32))


def _fallback(start_embedding, t_eval, W, b):
    M = W.T.astype(np.float64)
    bb = np.asarray(b, dtype=np.float64)
    y = start_embedding.astype(np.float64)
    t = np.asarray(t_eval, dtype=np.float64)
    traj = [y.copy()]
    for k in range(t.shape[0] - 1):
        h = (t[k+1] - t[k]) / 2.0
        for _ in range(2):
            y = _dopri5_step(y, h, M, bb)
        traj.append(y.copy())
    return np.stack(traj).astype(np.float32)


def _make_in_maps(y0, y1, E4_32):
    return [{"ya0": np.ascontiguousarray(y0[c*ROWS:(c+1)*ROWS, :]),
             "yb0": np.ascontiguousarray(y1[c*ROWS:(c+1)*ROWS, :]),
             "e4": E4_32} for c in range(N_CORES)]


def _assemble(y0, y1, results):
    out = np.empty((32, D, D), dtype=np.float32)
    out[0] = y0
    out[1] = y1
    for c in range(N_CORES):
        out[2:, c*ROWS:(c+1)*ROWS, :] = \
            results[c]["out"].reshape(OUT_BLOCKS, ROWS, D)
    return out


def kernel(start_embedding, t_eval, W, b):
    start_embedding = np.ascontiguousarray(start_embedding, dtype=np.float32)
    W32 = np.ascontiguousarray(W, dtype=np.float32)
    t = np.asarray(t_eval, dtype=np.float64)
    fast_ok = (start_embedding.shape == (D, D) and W32.shape == (D, D)
               and t.shape == (32,)
               and np.array_equal(t, np.arange(32, dtype=np.float64))
               and not np.any(np.asarray(b)))
    if not fast_ok:
        return _fallback(start_embedding, t_eval, W32, np.asarray(b))

    A, E4_32 = _host_propagators(W32)
    y1 = np.ascontiguousarray(
        (start_embedding.astype(np.float64) @ A).astype(np.float32))

    from concourse.bass_utils import run_bass_kernel_spmd
    nc = _get_nc()
    in_maps = _make_in_maps(start_embedding, y1, E4_32)
    res = run_bass_kernel_spmd(nc, in_maps, list(range(N_CORES)))
    return _assemble(start_embedding, y1, res.results)
